# revision 8
# baseline (speedup 1.0000x reference)
"""Trainium2 Bass kernel for the nonstationary Gaussian spectral-mixture kernel.

K[i,j] = sum_q wX[i,q] wY[j,q] * sqrt(2 sX sY)/s2 * exp(-d2/s2) * cos(2pi(phY-phX))

Sharding: rows of K (the x points) are split across 8 NeuronCores; y-side
features are computed redundantly on every core.

Per-pair decomposition used on device:
  d2      = nx + ny - 2 x.y                      (K=5 matmul)
  s2_q    = sX_q^2 (+) sY_q^2                    (K=2 outer-sum matmul)
  coef_q  = A_q (x) B_q + C_q (x) D_q            (K=2 outer-product matmul)
            A = wX sqrt(sX) cos(2pi phX), C = ... sin,
            B = wY sqrt(2 sY) cos(2pi phY), D = ... sin
  K       = sum_q coef_q * r * exp(-d2 * r),  r = 1/s2

The toolchain's ACT tables have no Softplus/Cos/Selu:
  softplus(z) = relu(z) + ln(1 + exp(-|z|))      (all in natural_log_exp set)
  sqrt(s)     = exp(0.5 ln(s))                   (avoids the sqrt table set)
  selu(z)     = lam*relu(z) + lam*alp*(exp(-relu(-z)) - 1)
  cos(t)      = sin(t + pi/2); all Sin ops are hoisted into one trig phase.
"""

import math

import numpy as np

N = 2048          # x points (rows of K)
M = 2048          # y points (cols of K)
D3 = 3
Q = 8
L = 64
NCORES = 8
NS = N // NCORES  # 256 x-points per core
P = 128           # partition tile
IT = NS // P      # 2 i-tiles per core
JC = 512          # j chunk (free dim)
NJ = M // JC      # 4 chunks

LAM = 1.0507009873554805    # selu lambda
ALPHA = 1.6732632423543772  # selu alpha

_CACHE = {}


def build():
    import concourse.bacc as bacc
    import concourse.tile as tile
    from concourse import mybir

    F32 = mybir.dt.float32
    AF = mybir.ActivationFunctionType
    OP = mybir.AluOpType
    AX = mybir.AxisListType

    nc = bacc.Bacc("TRN2", target_bir_lowering=False, debug=False,
                   num_devices=NCORES)

    def din(name, shape):
        return nc.dram_tensor(name, list(shape), F32, kind="ExternalInput").ap()

    a_xT = din("xT", (D3, NS))
    a_yT = din("yT", (D3, M))
    a_W1T = din("W1T", (D3, L))
    a_b1 = din("b1", (L, 1))
    # heads packed into one [72, n] block: f rows 0..23, w rows 32..39,
    # s rows 64..71 (32-aligned bases for downstream consumers); the lhsT
    # matrices are zero-padded so garbage rows are written as zeros.
    a_Wh0 = din("Wh0", (L, 32))   # [WfT | 0]        -> rows 0..31
    a_Wh1 = din("Wh1", (L, 32))   # [WwT | 0]        -> rows 32..63
    a_Wh2 = din("Wh2", (L, 8))    # WsT              -> rows 64..71
    a_bh = din("bh", (72, 1))     # packed bias (zeros on pad rows)
    a_bhN = din("bhN", (72, 1))   # -bh
    a_REP3 = din("REP3", (D3, 3 * Q))
    a_QD24 = din("QD24", (3 * Q, Q))
    a_ones3 = din("ones3", (D3, 1))
    a_ONES = din("ONESROW", (1, M))
    a_npi = din("negpi8", (Q, 1))
    a_out = nc.dram_tensor("out", [NS, M], F32, kind="ExternalOutput").ap()

    with tile.TileContext(nc) as tc:
        with tc.tile_pool(name="persist", bufs=1) as pp:
            # ---------- persistent SBUF ----------
            xT = pp.tile([D3, NS], F32, tag="xT")
            yT = pp.tile([D3, M], F32, tag="yT")
            W1T = pp.tile([D3, L], F32, tag="W1T")
            b1 = pp.tile([L, 1], F32, tag="b1")
            b1L = pp.tile([L, 1], F32, tag="b1L")
            b1N = pp.tile([L, 1], F32, tag="b1N")
            Wh0 = pp.tile([L, 32], F32, tag="Wh0")
            Wh1 = pp.tile([L, 32], F32, tag="Wh1")
            Wh2 = pp.tile([L, Q], F32, tag="Wh2")
            bh = pp.tile([72, 1], F32, tag="bh")
            bhN = pp.tile([72, 1], F32, tag="bhN")
            REP3 = pp.tile([D3, 3 * Q], F32, tag="REP3")
            QD24 = pp.tile([3 * Q, Q], F32, tag="QD24")
            ones3 = pp.tile([D3, 1], F32, tag="ones3")
            npi = pp.tile([Q, 1], F32, tag="npi")

            # pairwise matmul operands; row pairs at partitions (32k, 32k+1),
            # k = q % 4 ("a": q0-3, "b": q4-7)
            s2La = pp.tile([98, NS], F32, tag="s2La")    # (ones, aX_q)
            s2Lb = pp.tile([98, NS], F32, tag="s2Lb")
            cfLa = pp.tile([98, NS], F32, tag="cfLa")    # (A_q, C_q)
            cfLb = pp.tile([98, NS], F32, tag="cfLb")
            s2Ra = pp.tile([98, M], F32, tag="s2Ra")     # (aY_q, ones)
            s2Rb = pp.tile([98, M], F32, tag="s2Rb")
            cfRa = pp.tile([98, M], F32, tag="cfRa")     # (B_q, D_q)
            cfRb = pp.tile([98, M], F32, tag="cfRb")
            d2L = pp.tile([5, NS], F32, tag="d2L")       # x0,x1,x2,nx,ones
            d2R = pp.tile([5, M], F32, tag="d2R")        # -2y0,-2y1,-2y2,ones,ny

            # full-width carriers across the ACT-table phase boundary
            phX = pp.tile([Q, NS], F32, tag="phX")
            phY = pp.tile([Q, M], F32, tag="phY")
            ampX = pp.tile([Q, NS], F32, tag="ampX")
            ampY = pp.tile([Q, M], F32, tag="ampY")

            # ---------- load inputs ----------
            dma = nc.sync.dma_start
            dma(xT[:], a_xT)
            dma(yT[:], a_yT)
            dma(W1T[:], a_W1T)
            dma(b1[:], a_b1)
            dma(Wh0[:], a_Wh0)
            dma(Wh1[:], a_Wh1)
            dma(Wh2[:], a_Wh2)
            dma(bh[:], a_bh)
            dma(bhN[:], a_bhN)
            dma(REP3[:], a_REP3)
            dma(QD24[:], a_QD24)
            dma(ones3[:], a_ones3)
            dma(npi[:], a_npi)

            nc.scalar.mul(b1L[:], b1[:], LAM)
            nc.scalar.mul(b1N[:], b1[:], -1.0)

            # ones rows of the pairwise operands
            for k in range(4):
                dma(s2La[32 * k:32 * k + 1, :], a_ONES[:, :NS])
                dma(s2Lb[32 * k:32 * k + 1, :], a_ONES[:, :NS])
                dma(s2Ra[32 * k + 1:32 * k + 2, :], a_ONES)
                dma(s2Rb[32 * k + 1:32 * k + 2, :], a_ONES)
            dma(d2L[4:5, :], a_ONES[:, :NS])
            dma(d2R[3:4, :], a_ONES)
            dma(d2L[0:3, :], a_xT)

            # ---------- features, ln/exp table phase ----------
            with (
                tc.tile_pool(name="fscr", bufs=1) as fp_,
                tc.tile_pool(name="fpsum", bufs=1, space="PSUM") as fps,
            ):
                def feats(PT, n, is_y, aDa, aDb, aRow, phS, ampS, nrmD,
                          nrmRow):
                    nchunk = max(1, n // JC)
                    cw = n // nchunk
                    for c in range(nchunk):
                        sl = slice(c * cw, (c + 1) * cw)
                        # hT = selu(W1 @ p + b1).T  [L, cw]
                        hp = fps.tile([L, cw], F32, tag="hp")
                        nc.tensor.matmul(hp[:], W1T[:], PT[:, sl],
                                         start=True, stop=True)
                        hr = fp_.tile([L, cw], F32, tag="hr")
                        nc.scalar.activation(hr[:], hp[:], AF.Relu,
                                             bias=b1L[:], scale=LAM)
                        hn = fp_.tile([L, cw], F32, tag="hn")
                        nc.scalar.activation(hn[:], hp[:], AF.Relu,
                                             bias=b1N[:], scale=-1.0)
                        he = fp_.tile([L, cw], F32, tag="he")
                        nc.scalar.activation(he[:], hn[:], AF.Exp, scale=-1.0)
                        ht1 = fp_.tile([L, cw], F32, tag="ht1")
                        nc.vector.tensor_scalar(ht1[:], he[:], LAM * ALPHA,
                                                -LAM * ALPHA, OP.mult, OP.add)
                        hT = fp_.tile([L, cw], F32, tag="hT")
                        nc.vector.tensor_add(hT[:], ht1[:], hr[:])

                        # heads: one [72, cw] PSUM block, softplus via
                        # relu(z+b) + ln(1 + exp(-|z+b|))
                        hd = fps.tile([72, cw], F32, tag="hd")
                        nc.tensor.matmul(hd[0:32, :], Wh0[:], hT[:],
                                         start=True, stop=True)
                        nc.tensor.matmul(hd[32:64, :], Wh1[:], hT[:],
                                         start=True, stop=True,
                                         tile_position=(0, 32))
                        nc.tensor.matmul(hd[64:72, :], Wh2[:], hT[:],
                                         start=True, stop=True,
                                         tile_position=(0, 64))
                        az = fp_.tile([72, cw], F32, tag="az")
                        nc.scalar.activation(az[:], hd[:], AF.Abs, bias=bh[:])
                        e1 = fp_.tile([72, cw], F32, tag="e1")
                        nc.scalar.activation(e1[:], az[:], AF.Exp, scale=-1.0)
                        l1 = fp_.tile([72, cw], F32, tag="l1")
                        nc.scalar.activation(l1[:], e1[:], AF.Ln, bias=1.0)
                        r1 = fp_.tile([72, cw], F32, tag="r1")
                        nc.scalar.activation(r1[:], hd[:], AF.Relu, bias=bh[:])
                        sp = fp_.tile([72, cw], F32, tag="sp")
                        nc.vector.tensor_add(sp[:], l1[:], r1[:])
                        # rows: f = sp[0:24], w = sp[32:40], s = sp[64:72].
                        # walrus requires equal SB base partitions for
                        # tensor_tensor operands -> move w/s rows to base 0.
                        wv = fp_.tile([Q, cw], F32, tag="wv")
                        dma(wv[:], sp[32:40, :])
                        sv = fp_.tile([Q, cw], F32, tag="sv")
                        dma(sv[:], sp[64:72, :])

                        # aP = s^2 -> scattered into the s2 operand rows
                        aP = fp_.tile([Q, cw], F32, tag="aP")
                        nc.scalar.activation(aP[:], sv[:], AF.Square)
                        for k in range(4):
                            r0 = 32 * k + aRow
                            dma(aDa[r0:r0 + 1, sl], aP[k:k + 1, :])
                            dma(aDb[r0:r0 + 1, sl], aP[4 + k:5 + k, :])

                        # amp = w * sqrt((2)s) = w * exp(0.5 ln((2)s))
                        lnc = fp_.tile([Q, cw], F32, tag="lnc")
                        nc.scalar.activation(lnc[:], sv[:], AF.Ln,
                                             scale=2.0 if is_y else 1.0)
                        ssq = fp_.tile([Q, cw], F32, tag="ssq")
                        nc.scalar.activation(ssq[:], lnc[:], AF.Exp, scale=0.5)
                        nc.vector.tensor_mul(ampS[:, sl], wv[:], ssq[:])

                        # phase = einsum('qd,d->q', f, p)
                        ptr = fps.tile([3 * Q, cw], F32, tag="ptr")
                        nc.tensor.matmul(ptr[:], REP3[:], PT[:, sl],
                                         start=True, stop=True)
                        fpm = fp_.tile([3 * Q, cw], F32, tag="fpm")
                        nc.vector.tensor_mul(fpm[:], sp[0:24, :], ptr[:])
                        php = fps.tile([Q, cw], F32, tag="php")
                        nc.tensor.matmul(php[:], QD24[:], fpm[:],
                                         start=True, stop=True)
                        nc.scalar.copy(phS[:, sl], php[:])

                        # |p|^2 row
                        sqP = fp_.tile([D3, cw], F32, tag="sqP")
                        nc.scalar.activation(sqP[:], PT[:, sl], AF.Square)
                        nrm = fps.tile([1, cw], F32, tag="nrm")
                        nc.tensor.matmul(nrm[:], ones3[:], sqP[:],
                                         start=True, stop=True)
                        nrs = fp_.tile([1, cw], F32, tag="nrs")
                        nc.scalar.copy(nrs[:], nrm[:])
                        dma(nrmD[nrmRow:nrmRow + 1, sl], nrs[:])

                        if is_y:
                            ym2 = fp_.tile([D3, cw], F32, tag="ym2")
                            nc.scalar.mul(ym2[:], PT[:, sl], -2.0)
                            dma(d2R[0:3, sl], ym2[:])

                feats(xT, NS, False, s2La, s2Lb, 1, phX, ampX, d2L, 3)
                feats(yT, M, True, s2Ra, s2Rb, 0, phY, ampY, d2R, 4)

            # ---------- trig phase: A/B = amp*cos, C/D = amp*sin ----------
            with tc.tile_pool(name="trig", bufs=1) as tg:
                for (phS, ampS, n, cDa, cDb) in (
                    (phX, ampX, NS, cfLa, cfLb),
                    (phY, ampY, M, cfRa, cfRb),
                ):
                    # ACT Sin is only valid on [-pi, pi]: range-reduce via
                    # u = (ph + c) - rint(ph + c) in [-0.5, 0.5] using the
                    # fp32 round-to-int magic constant; sin(2pi u) equals
                    # sin(2pi(ph + c)); c = 0.25 gives cos, c = 0 gives sin.
                    MAGIC = 12582912.0  # 1.5 * 2**23
                    uc = tg.tile([Q, n], F32, tag="uc")
                    us = tg.tile([Q, n], F32, tag="us")
                    for (u, cc) in ((uc, 0.25), (us, 0.0)):
                        t1 = tg.tile([Q, n], F32, tag="t1")
                        nc.vector.tensor_scalar(t1[:], phS[:], cc, MAGIC,
                                                OP.add, OP.add)
                        t2 = tg.tile([Q, n], F32, tag="t2")
                        nc.vector.tensor_scalar(t2[:], t1[:], -MAGIC, None,
                                                OP.add)
                        nc.vector.scalar_tensor_tensor(
                            u[:], phS[:], cc, t2[:], OP.add, OP.subtract)
                    cosv = tg.tile([Q, n], F32, tag="cosv")
                    nc.scalar.activation(cosv[:], uc[:], AF.Sin,
                                         scale=2.0 * math.pi)
                    sinv = tg.tile([Q, n], F32, tag="sinv")
                    nc.scalar.activation(sinv[:], us[:], AF.Sin,
                                         scale=2.0 * math.pi)
                    Av = tg.tile([Q, n], F32, tag="Av")
                    nc.vector.tensor_mul(Av[:], ampS[:], cosv[:])
                    Cv = tg.tile([Q, n], F32, tag="Cv")
                    nc.vector.tensor_mul(Cv[:], ampS[:], sinv[:])
                    for k in range(4):
                        r0 = 32 * k
                        dma(cDa[r0:r0 + 1, :], Av[k:k + 1, :])
                        dma(cDb[r0:r0 + 1, :], Av[4 + k:5 + k, :])
                        dma(cDa[r0 + 1:r0 + 2, :], Cv[k:k + 1, :])
                        dma(cDb[r0 + 1:r0 + 2, :], Cv[4 + k:5 + k, :])

            # ---------- pairwise stage ----------
            with (
                tc.tile_pool(name="pw", bufs=3) as pw,
                tc.tile_pool(name="mqp", bufs=1) as mqp,
                tc.tile_pool(name="accp", bufs=2) as accp,
                tc.tile_pool(name="d2p", bufs=2, space="PSUM") as d2p,
                tc.tile_pool(name="scp", bufs=4, space="PSUM") as scp,
            ):
                for it in range(IT):
                    isl = slice(it * P, (it + 1) * P)
                    for c in range(NJ):
                        jsl = slice(c * JC, (c + 1) * JC)
                        d2ps = d2p.tile([P, JC], F32, tag="d2")
                        nc.tensor.matmul(d2ps[:], d2L[0:5, isl],
                                         d2R[0:5, jsl], start=True, stop=True)
                        mq = mqp.tile([P, Q * JC], F32, tag="mq")
                        for q in range(Q):
                            r0 = 32 * (q % 4)
                            sL = s2La if q < 4 else s2Lb
                            sR = s2Ra if q < 4 else s2Rb
                            cL = cfLa if q < 4 else cfLb
                            cR = cfRa if q < 4 else cfRb
                            s2ps = scp.tile([P, JC], F32, tag="sc")
                            nc.tensor.matmul(s2ps[:], sL[r0:r0 + 2, isl],
                                             sR[r0:r0 + 2, jsl], start=True,
                                             stop=True, tile_position=(r0, 0))
                            cfps = scp.tile([P, JC], F32, tag="sc")
                            nc.tensor.matmul(cfps[:], cL[r0:r0 + 2, isl],
                                             cR[r0:r0 + 2, jsl], start=True,
                                             stop=True, tile_position=(r0, 0))
                            rq = pw.tile([P, JC], F32, tag="rq")
                            nc.vector.reciprocal_approx_fast(out=rq[:],
                                                             in_=s2ps[:])
                            tq = pw.tile([P, JC], F32, tag="tq")
                            nc.vector.tensor_mul(tq[:], d2ps[:], rq[:])
                            eq = pw.tile([P, JC], F32, tag="eq")
                            nc.scalar.activation(eq[:], tq[:], AF.Exp,
                                                 scale=-1.0)
                            m1 = pw.tile([P, JC], F32, tag="m1")
                            nc.vector.tensor_mul(m1[:], cfps[:], rq[:])
                            nc.vector.tensor_mul(mq[:, q * JC:(q + 1) * JC],
                                                 m1[:], eq[:])
                        acc = accp.tile([P, JC], F32, tag="acc")
                        nc.vector.tensor_reduce(
                            acc[:], mq[:].rearrange("p (q j) -> p j q", q=Q),
                            axis=AX.X, op=OP.add)
                        nc.sync.dma_start(a_out[isl, jsl], acc[:])

    nc.compile()
    return nc


def _get_nc():
    if "nc" not in _CACHE:
        _CACHE["nc"] = build()
    return _CACHE["nc"]


def make_in_maps(x, y, W1, b1, Ww, bw, Wf, bf, Ws, bs):
    f = np.float32
    Wh0 = np.zeros((L, 32), dtype=f)
    Wh0[:, :24] = np.asarray(Wf, dtype=f).T
    Wh1 = np.zeros((L, 32), dtype=f)
    Wh1[:, :8] = np.asarray(Ww, dtype=f).T
    Wh2 = np.ascontiguousarray(np.asarray(Ws, dtype=f).T)
    bhp = np.zeros((72, 1), dtype=f)
    bhp[0:24, 0] = np.asarray(bf, dtype=f)
    bhp[32:40, 0] = np.asarray(bw, dtype=f)
    bhp[64:72, 0] = np.asarray(bs, dtype=f)
    common = {
        "yT": np.ascontiguousarray(np.asarray(y, dtype=f).T),
        "W1T": np.ascontiguousarray(np.asarray(W1, dtype=f).T),
        "b1": np.ascontiguousarray(np.asarray(b1, dtype=f).reshape(L, 1)),
        "Wh0": Wh0,
        "Wh1": Wh1,
        "Wh2": Wh2,
        "bh": bhp,
        "bhN": -bhp,
        "REP3": np.ascontiguousarray(np.tile(np.eye(D3, dtype=f), (1, Q))),
        "QD24": np.ascontiguousarray(np.repeat(np.eye(Q, dtype=f), D3,
                                               axis=0)),
        "ones3": np.ones((D3, 1), dtype=f),
        "ONESROW": np.ones((1, M), dtype=f),
        "negpi8": np.full((Q, 1), -math.pi, dtype=f),
    }
    in_maps = []
    for c in range(NCORES):
        m = dict(common)
        m["xT"] = np.ascontiguousarray(np.asarray(x, dtype=f)
                                       [c * NS:(c + 1) * NS].T)
        in_maps.append(m)
    return in_maps


def kernel(x, y, W1, b1, Ww, bw, Wf, bf, Ws, bs):
    from concourse import bass_utils

    nc = _get_nc()
    in_maps = make_in_maps(x, y, W1, b1, Ww, bw, Wf, bf, Ws, bs)
    res = bass_utils.run_bass_kernel_spmd(nc, in_maps,
                                          core_ids=list(range(NCORES)))
    return np.concatenate([res.results[c]["out"] for c in range(NCORES)],
                          axis=0)


if __name__ == "__main__":
    d = np.load("/root/problem/inputs_cache.npz")
    out = kernel(**{k: d[k] for k in d.files})
    print("out", out.shape, out.dtype, float(np.abs(out).max()))
